# revision 15
# baseline (speedup 1.0000x reference)
"""ArcFace-style angular-penalty softmax loss on 8 TRN2 NeuronCores.

Strategy (class/tensor-parallel partial-FC):
  - W [50000, 512] sharded along classes (6250/core), pre-transposed to
    WT [512, 6250] and cast to fp8e4m3 on the host; x replicated both as
    xT [512, 2048] fp8 (matmul stationary) and natural [2048, 512] bf16
    (row norms).
  - TensorEngine computes raw logits with fp8 DoubleRow matmuls
    (contracting 256 rows per instruction) into 4-bank f32 PSUM chunks.
  - ScalarEngine fuses exp(S * inv_norm[n] * raw) with the per-row sum
    (per-partition scale AP + accum_out), writing bf16 exp values.
  - VectorEngine extracts exp(S*inv_norm*target) with one
    scalar_tensor_tensor per chunk: (iota == label) * expvals, accum.
  - One AllReduce of [128, 32] f32 per-row partials (exp sums + target
    exps), then every core computes the identical scalar loss.
"""

import numpy as np
import ml_dtypes

N, D, C = 2048, 512, 50000
NCORES = 8
CLOC = C // NCORES          # 6250 classes per core
S, MARGIN, EPS = 64.0, 0.5, 1e-7
NT = N // 128               # 16 row tiles
CHUNK = 2048                # classes per PSUM chunk (4 banks of f32)
JC = [(j * CHUNK, min(CHUNK, CLOC - j * CHUNK))
      for j in range((CLOC + CHUNK - 1) // CHUNK)]  # [(0,2048)x3, (6144,106)]
NJ = len(JC)

_COMPILED = {}


def _build():
    from concourse import bass, bacc, tile, mybir

    f32 = mybir.dt.float32
    bf16 = mybir.dt.bfloat16
    fp8 = mybir.dt.float8e4
    i16 = mybir.dt.int16
    Alu = mybir.AluOpType
    Act = mybir.ActivationFunctionType
    DR = mybir.MatmulPerfMode.DoubleRow

    nc = bacc.Bacc("TRN2", target_bir_lowering=False, debug=False,
                   num_devices=NCORES)

    # packed layouts ([128, cols] with per-slab column blocks -> 1 DMA each)
    # xt cols: d2-major [d2][dd][n];  wt cols: jc-major [jc][d2][dd][c]
    # xn cols: [i][d]
    xt_d = nc.dram_tensor("xt", [128, 4 * N], fp8, kind="ExternalInput")
    xn_d = nc.dram_tensor("xn", [128, NT * D], bf16, kind="ExternalInput")
    wt_d = nc.dram_tensor("wt", [128, 4 * CLOC], fp8, kind="ExternalInput")
    lab_d = nc.dram_tensor("lab", [128, NT], f32, kind="ExternalInput")
    out_d = nc.dram_tensor("out", [1, 1], f32, kind="ExternalOutput")

    with tile.TileContext(nc) as tc:
        with (
            tc.tile_pool(name="big", bufs=1) as big,
            tc.tile_pool(name="scr", bufs=3) as scr,
            tc.tile_pool(name="psum", bufs=2, space="PSUM") as psum,
            tc.tile_pool(name="dram", bufs=1, space="DRAM") as dram,
        ):
            # stationary x.T: two [128, 2, N] fp8 slabs (one per DoubleRow pair)
            xt_sb = [big.tile([128, 2, N], fp8, name=f"xt_sb{d2}")
                     for d2 in range(2)]
            # moving W.T: per (d2, chunk) fp8 slabs
            wt_sb = {}
            for d2 in range(2):
                for jc, (c0, cw) in enumerate(JC):
                    wt_sb[d2, jc] = big.tile([128, 2, cw], fp8,
                                             name=f"wt_sb{d2}_{jc}")
            xn_sb = [big.tile([128, 4, D], bf16, name=f"xn_sb{q}")
                     for q in range(4)]
            lab_sb = big.tile([128, NT], f32, name="lab_sb")
            iota_sb = big.tile([128, CLOC], i16, name="iota_sb")

            sumsq = big.tile([128, NT], f32, name="sumsq")
            sinv = big.tile([128, NT], f32, name="sinv")
            invn = big.tile([128, NT], f32, name="invn")
            sums_f = big.tile([128, NT * NJ], f32, name="sums_f")
            texp_f = big.tile([128, NT * NJ], f32, name="texp_f")
            stats = big.tile([128, 32], f32, name="stats")
            allred = big.tile([128, 32], f32, name="allred")
            ones = big.tile([128, 1], f32, name="ones")

            # DMAs, in first-use order; xn/lab issue from gpsimd so they
            # don't serialize behind the weight loads on sync
            for d2 in range(2):
                nc.sync.dma_start(out=xt_sb[d2][:, :, :],
                                  in_=xt_d[:, d2 * 2 * N:(d2 + 1) * 2 * N])
            nc.gpsimd.dma_start(out=lab_sb[:, :], in_=lab_d[:, :])
            for q in range(4):
                nc.gpsimd.dma_start(
                    out=xn_sb[q][:, :, :],
                    in_=xn_d[:, q * 4 * D:(q + 1) * 4 * D])
            boff = 0
            for jc, (c0, cw) in enumerate(JC):
                for d2 in range(2):
                    if jc == 0:
                        # halves so the first matmuls unblock sooner
                        h = cw // 2
                        blk = wt_d[:, boff:boff + 2 * cw].rearrange(
                            "p (dd c) -> p dd c", dd=2)
                        nc.sync.dma_start(out=wt_sb[d2, jc][:, :, 0:h],
                                          in_=blk[:, :, 0:h])
                        nc.sync.dma_start(out=wt_sb[d2, jc][:, :, h:cw],
                                          in_=blk[:, :, h:cw])
                    else:
                        nc.sync.dma_start(
                            out=wt_sb[d2, jc][:, :, :],
                            in_=wt_d[:, boff:boff + 2 * cw])
                    boff += 2 * cw
            nc.gpsimd.iota(iota_sb[:, :], pattern=[[1, CLOC]], base=0,
                           channel_multiplier=0)
            nc.vector.memset(ones[:, :], 1.0)

            # row sum-of-squares -> S / ||x_n|| (per quarter, so the first
            # exp tiles unblock before all of x has landed)
            for q in range(4):
                for iq in range(4):
                    i = q * 4 + iq
                    junk = scr.tile([128, D], bf16, tag="sqjunk",
                                    name="sqjunk")
                    nc.vector.scalar_tensor_tensor(
                        out=junk[:, :], in0=xn_sb[q][:, iq, :], scalar=0.0,
                        in1=xn_sb[q][:, iq, :], op0=Alu.add, op1=Alu.mult,
                        accum_out=sumsq[:, i:i + 1])
                sl = slice(q * 4, (q + 1) * 4)
                nc.scalar.activation(out=sinv[:, sl], in_=sumsq[:, sl],
                                     func=Act.Sqrt)
                nc.vector.reciprocal(out=invn[:, sl], in_=sinv[:, sl])
                nc.vector.tensor_scalar_mul(sinv[:, sl], invn[:, sl],
                                            float(S))

            # main loop
            for i in range(NT):
                for jc, (c0, cw) in enumerate(JC):
                    pt = psum.tile([128, cw], f32, tag="mm",
                                   name=f"mm_{i}_{jc}")
                    for h0 in range(0, cw, 512):
                        hw = min(512, cw - h0)
                        for d2 in range(2):
                            nc.tensor.matmul(
                                pt[:, h0:h0 + hw],
                                lhsT=xt_sb[d2][:, :, i * 128:(i + 1) * 128],
                                rhs=wt_sb[d2, jc][:, :, h0:h0 + hw],
                                start=(d2 == 0), stop=(d2 == 1),
                                perf_mode=DR)
                    ej = scr.tile([128, CHUNK], bf16, tag="ej", name="ej",
                                  bufs=6)
                    nc.scalar.activation(
                        out=ej[:, :cw], in_=pt[:, :], func=Act.Exp,
                        scale=sinv[:, i:i + 1],
                        accum_out=sums_f[:, i * NJ + jc:i * NJ + jc + 1])
                    tj = scr.tile([128, CHUNK], bf16, tag="tj", name="tj",
                                  bufs=2)
                    nc.vector.scalar_tensor_tensor(
                        out=tj[:, :cw], in0=iota_sb[:, c0:c0 + cw],
                        scalar=lab_sb[:, i:i + 1], in1=ej[:, :cw],
                        op0=Alu.is_equal, op1=Alu.mult,
                        accum_out=texp_f[:, i * NJ + jc:i * NJ + jc + 1])
                # fold row i's chunk partials as soon as they exist
                nc.vector.tensor_reduce(
                    out=stats[:, i:i + 1], in_=sums_f[:, i * NJ:(i + 1) * NJ],
                    axis=mybir.AxisListType.X, op=Alu.add)
                nc.vector.tensor_reduce(
                    out=stats[:, NT + i:NT + i + 1],
                    in_=texp_f[:, i * NJ:(i + 1) * NJ],
                    axis=mybir.AxisListType.X, op=Alu.add)

            cc_in = dram.tile([128, 32], f32, name="cc_in")
            cc_out = dram.tile([128, 32], f32, name="cc_out",
                               addr_space="Shared")
            nc.sync.dma_start(out=cc_in[:, :], in_=stats[:, :])
            nc.gpsimd.collective_compute(
                "AllReduce", Alu.add,
                replica_groups=[list(range(NCORES))],
                ins=[cc_in[:, :].opt()], outs=[cc_out[:, :].opt()])
            nc.sync.dma_start(out=allred[:, :], in_=cc_out[:, :])

            # final scalar math on [128, NT]
            sums_g = allred[:, 0:NT]
            texp_g = allred[:, NT:2 * NT]
            tln = big.tile([128, NT], f32, name="tln")
            nc.scalar.activation(out=tln[:, :], in_=texp_g, func=Act.Ln)
            tgt = big.tile([128, NT], f32, name="tgt")
            nc.vector.tensor_scalar_mul(tgt[:, :], tln[:, :], float(1.0 / S))
            t = big.tile([128, NT], f32, name="t")
            nc.vector.tensor_scalar(out=t[:, :], in0=tgt[:, :],
                                    scalar1=float(1.0 - EPS),
                                    scalar2=float(-1.0 + EPS),
                                    op0=Alu.min, op1=Alu.max)
            tsq = big.tile([128, NT], f32, name="tsq")
            nc.vector.tensor_mul(tsq[:, :], t[:, :], t[:, :])
            u = big.tile([128, NT], f32, name="u")
            nc.vector.tensor_scalar(out=u[:, :], in0=tsq[:, :],
                                    scalar1=-1.0, scalar2=1.0,
                                    op0=Alu.mult, op1=Alu.add)
            r = big.tile([128, NT], f32, name="r")
            nc.scalar.activation(out=r[:, :], in_=u[:, :], func=Act.Sqrt)
            rs = big.tile([128, NT], f32, name="rs")
            nc.vector.tensor_scalar_mul(rs[:, :], r[:, :],
                                        float(S * np.sin(MARGIN)))
            numer = big.tile([128, NT], f32, name="numer")
            nc.vector.scalar_tensor_tensor(
                out=numer[:, :], in0=t[:, :], scalar=float(S * np.cos(MARGIN)),
                in1=rs[:, :], op0=Alu.mult, op1=Alu.subtract)
            en = big.tile([128, NT], f32, name="en")
            nc.scalar.activation(out=en[:, :], in_=numer[:, :], func=Act.Exp)
            d1 = big.tile([128, NT], f32, name="d1")
            nc.vector.tensor_add(d1[:, :], en[:, :], sums_g)
            denom = big.tile([128, NT], f32, name="denom")
            nc.vector.tensor_sub(denom[:, :], d1[:, :], texp_g)
            ld = big.tile([128, NT], f32, name="ld")
            nc.scalar.activation(out=ld[:, :], in_=denom[:, :], func=Act.Ln)
            L = big.tile([128, NT], f32, name="L")
            nc.vector.tensor_sub(L[:, :], numer[:, :], ld[:, :])
            Lrow = big.tile([128, 1], f32, name="Lrow")
            nc.vector.tensor_reduce(out=Lrow[:, :], in_=L[:, :],
                                    axis=mybir.AxisListType.X, op=Alu.add)
            acc = psum.tile([1, 1], f32, tag="mm", name="acc")
            nc.tensor.matmul(acc[:, :], lhsT=Lrow[:, :], rhs=ones[:, :])
            fin = big.tile([1, 1], f32, name="fin")
            nc.scalar.activation(out=fin[:, :], in_=acc[:, :], func=Act.Copy,
                                 scale=float(-1.0 / N))
            nc.sync.dma_start(out=out_d[:, :], in_=fin[:, :])

    nc.compile()
    return nc


def _get_nc():
    if "nc" not in _COMPILED:
        _COMPILED["nc"] = _build()
    return _COMPILED["nc"]


def make_in_maps(x, labels, W):
    x = np.asarray(x, np.float32)
    labels = np.asarray(labels, np.int64)
    W = np.asarray(W, np.float32)

    # xt packed [128, 4N]: col d2*2N + dd*N + n <- x[n, (2*d2+dd)*128 + p]
    xtr = x.T.reshape(4, 128, N)                  # [drow, p, n]
    xt = np.ascontiguousarray(
        xtr.reshape(2, 2, 128, N).transpose(2, 0, 1, 3).reshape(128, 4 * N)
    ).astype(ml_dtypes.float8_e4m3)
    # xn packed [128, NT*D]: col i*D + d <- x[i*128 + p, d]
    xn = np.ascontiguousarray(
        x.reshape(NT, 128, D).transpose(1, 0, 2).reshape(128, NT * D)
    ).astype(ml_dtypes.bfloat16)

    in_maps = []
    for k in range(NCORES):
        lo = k * CLOC
        wtt = W[lo:lo + CLOC].T                   # [D, CLOC] = [drow*? , c]
        blocks = []
        for (c0, cw) in JC:
            blk = wtt[:, c0:c0 + cw].reshape(2, 2, 128, cw)  # [d2, dd, p, c]
            blocks.append(blk.transpose(0, 2, 1, 3).reshape(2, 128, 2 * cw))
        wt = np.ascontiguousarray(
            np.concatenate([b for blk2 in blocks for b in blk2], axis=1)
        ).astype(ml_dtypes.float8_e4m3)
        ll = labels - lo
        ll = np.where((ll >= 0) & (ll < CLOC), ll, -1).astype(np.float32)
        lab = np.ascontiguousarray(ll.reshape(NT, 128).T)  # [128, NT]
        in_maps.append({"xt": xt, "xn": xn, "wt": wt, "lab": lab})
    return in_maps


def kernel(x, labels, W, _trace=False, _trace_kwargs=None):
    from concourse.bass_utils import run_bass_kernel_spmd

    nc = _get_nc()
    in_maps = make_in_maps(x, labels, W)
    res = run_bass_kernel_spmd(nc, in_maps, core_ids=list(range(NCORES)),
                               trace=_trace, **(_trace_kwargs or {}))
    if _trace:
        _COMPILED["last_result"] = res
    out = np.asarray(res.results[0]["out"], np.float32).reshape(())
    return out


# revision 16
# speedup vs baseline: 1.1488x; 1.1488x over previous
"""ArcFace-style angular-penalty softmax loss on 8 TRN2 NeuronCores.

Strategy (class/tensor-parallel partial-FC):
  - W [50000, 512] sharded along classes (6250/core), pre-transposed to
    WT [512, 6250] and cast to fp8e4m3 on the host; x replicated both as
    xT [512, 2048] fp8 (matmul stationary) and natural [2048, 512] bf16
    (row norms).
  - TensorEngine computes raw logits with fp8 DoubleRow matmuls
    (contracting 256 rows per instruction) into 4-bank f32 PSUM chunks.
  - ScalarEngine fuses exp(S * inv_norm[n] * raw) with the per-row sum
    (per-partition scale AP + accum_out), writing bf16 exp values.
  - VectorEngine extracts exp(S*inv_norm*target) with one
    scalar_tensor_tensor per chunk: (iota == label) * expvals, accum.
  - One AllReduce of [128, 32] f32 per-row partials (exp sums + target
    exps), then every core computes the identical scalar loss.
"""

import numpy as np
import ml_dtypes

N, D, C = 2048, 512, 50000
NCORES = 8
CLOC = C // NCORES          # 6250 classes per core
S, MARGIN, EPS = 64.0, 0.5, 1e-7
NT = N // 128               # 16 row tiles
CHUNK = 2048                # classes per PSUM chunk (4 banks of f32)
JC = [(j * CHUNK, min(CHUNK, CLOC - j * CHUNK))
      for j in range((CLOC + CHUNK - 1) // CHUNK)]  # [(0,2048)x3, (6144,106)]
NJ = len(JC)

_COMPILED = {}


def _build():
    from concourse import bass, bacc, tile, mybir

    f32 = mybir.dt.float32
    bf16 = mybir.dt.bfloat16
    fp8 = mybir.dt.float8e4
    i16 = mybir.dt.int16
    Alu = mybir.AluOpType
    Act = mybir.ActivationFunctionType
    DR = mybir.MatmulPerfMode.DoubleRow

    nc = bacc.Bacc("TRN2", target_bir_lowering=False, debug=False,
                   num_devices=NCORES)

    # packed layouts ([128, cols] with per-slab column blocks -> 1 DMA each)
    # xt cols: d2-major [d2][dd][n];  wt cols: jc-major [jc][d2][dd][c]
    # xn cols: [i][d]
    xt_d = nc.dram_tensor("xt", [128, 4 * N], fp8, kind="ExternalInput")
    xn_d = nc.dram_tensor("xn", [128, NT * D], bf16, kind="ExternalInput")
    wt_d = nc.dram_tensor("wt", [128, 4 * CLOC], fp8, kind="ExternalInput")
    lab_d = nc.dram_tensor("lab", [128, NT], f32, kind="ExternalInput")
    out_d = nc.dram_tensor("out", [1, 1], f32, kind="ExternalOutput")

    with tile.TileContext(nc) as tc:
        with (
            tc.tile_pool(name="big", bufs=1) as big,
            tc.tile_pool(name="scr", bufs=3) as scr,
            tc.tile_pool(name="psum", bufs=2, space="PSUM") as psum,
            tc.tile_pool(name="dram", bufs=1, space="DRAM") as dram,
        ):
            # stationary x.T: two [128, 2, N] fp8 slabs (one per DoubleRow pair)
            xt_sb = [big.tile([128, 2, N], fp8, name=f"xt_sb{d2}")
                     for d2 in range(2)]
            # moving W.T: per (d2, chunk) fp8 slabs
            wt_sb = {}
            for d2 in range(2):
                for jc, (c0, cw) in enumerate(JC):
                    wt_sb[d2, jc] = big.tile([128, 2, cw], fp8,
                                             name=f"wt_sb{d2}_{jc}")
            xn_sb = [big.tile([128, 4, D], bf16, name=f"xn_sb{q}")
                     for q in range(4)]
            lab_sb = big.tile([128, NT], f32, name="lab_sb")
            iota_sb = big.tile([128, CLOC], i16, name="iota_sb")

            sumsq = big.tile([128, NT], f32, name="sumsq")
            sinv = big.tile([128, NT], f32, name="sinv")
            invn = big.tile([128, NT], f32, name="invn")
            sums_f = big.tile([128, NT * NJ], f32, name="sums_f")
            texp_f = big.tile([128, NT * NJ], f32, name="texp_f")
            stats = big.tile([128, 32], f32, name="stats")
            allred = big.tile([128, 32], f32, name="allred")
            ones = big.tile([128, 1], f32, name="ones")

            # DMAs, in first-use order; xn/lab issue from gpsimd so they
            # don't serialize behind the weight loads on sync
            for d2 in range(2):
                nc.sync.dma_start(out=xt_sb[d2][:, :, :],
                                  in_=xt_d[:, d2 * 2 * N:(d2 + 1) * 2 * N])
            nc.gpsimd.dma_start(out=lab_sb[:, :], in_=lab_d[:, :])
            for q in range(4):
                nc.gpsimd.dma_start(
                    out=xn_sb[q][:, :, :],
                    in_=xn_d[:, q * 4 * D:(q + 1) * 4 * D])
            boff = 0
            for jc, (c0, cw) in enumerate(JC):
                for d2 in range(2):
                    nc.sync.dma_start(
                        out=wt_sb[d2, jc][:, :, :],
                        in_=wt_d[:, boff:boff + 2 * cw])
                    boff += 2 * cw
            nc.gpsimd.iota(iota_sb[:, :], pattern=[[1, CLOC]], base=0,
                           channel_multiplier=0)
            nc.vector.memset(ones[:, :], 1.0)

            # row sum-of-squares -> S / ||x_n|| (per quarter, so the first
            # exp tiles unblock before all of x has landed)
            for q in range(4):
                for iq in range(4):
                    i = q * 4 + iq
                    junk = scr.tile([128, D], bf16, tag="sqjunk",
                                    name="sqjunk")
                    nc.vector.scalar_tensor_tensor(
                        out=junk[:, :], in0=xn_sb[q][:, iq, :], scalar=0.0,
                        in1=xn_sb[q][:, iq, :], op0=Alu.add, op1=Alu.mult,
                        accum_out=sumsq[:, i:i + 1])
                sl = slice(q * 4, (q + 1) * 4)
                nc.scalar.activation(out=sinv[:, sl], in_=sumsq[:, sl],
                                     func=Act.Sqrt)
                nc.vector.reciprocal(out=invn[:, sl], in_=sinv[:, sl])
                nc.vector.tensor_scalar_mul(sinv[:, sl], invn[:, sl],
                                            float(S))

            # main loop
            for i in range(NT):
                for jc, (c0, cw) in enumerate(JC):
                    pt = psum.tile([128, cw], f32, tag="mm",
                                   name=f"mm_{i}_{jc}")
                    for h0 in range(0, cw, 512):
                        hw = min(512, cw - h0)
                        for d2 in range(2):
                            nc.tensor.matmul(
                                pt[:, h0:h0 + hw],
                                lhsT=xt_sb[d2][:, :, i * 128:(i + 1) * 128],
                                rhs=wt_sb[d2, jc][:, :, h0:h0 + hw],
                                start=(d2 == 0), stop=(d2 == 1),
                                perf_mode=DR)
                    ej = scr.tile([128, CHUNK], bf16, tag="ej", name="ej")
                    nc.scalar.activation(
                        out=ej[:, :cw], in_=pt[:, :], func=Act.Exp,
                        scale=sinv[:, i:i + 1],
                        accum_out=sums_f[:, i * NJ + jc:i * NJ + jc + 1])
                    tj = scr.tile([128, CHUNK], bf16, tag="tj", name="tj")
                    nc.vector.scalar_tensor_tensor(
                        out=tj[:, :cw], in0=iota_sb[:, c0:c0 + cw],
                        scalar=lab_sb[:, i:i + 1], in1=ej[:, :cw],
                        op0=Alu.is_equal, op1=Alu.mult,
                        accum_out=texp_f[:, i * NJ + jc:i * NJ + jc + 1])

            # fold chunk partials; pack [sums | texp] -> stats
            for i in range(NT):
                nc.vector.tensor_reduce(
                    out=stats[:, i:i + 1], in_=sums_f[:, i * NJ:(i + 1) * NJ],
                    axis=mybir.AxisListType.X, op=Alu.add)
                nc.vector.tensor_reduce(
                    out=stats[:, NT + i:NT + i + 1],
                    in_=texp_f[:, i * NJ:(i + 1) * NJ],
                    axis=mybir.AxisListType.X, op=Alu.add)

            cc_in = dram.tile([128, 32], f32, name="cc_in")
            cc_out = dram.tile([128, 32], f32, name="cc_out",
                               addr_space="Shared")
            nc.sync.dma_start(out=cc_in[:, :], in_=stats[:, :])
            nc.gpsimd.collective_compute(
                "AllReduce", Alu.add,
                replica_groups=[list(range(NCORES))],
                ins=[cc_in[:, :].opt()], outs=[cc_out[:, :].opt()])
            nc.sync.dma_start(out=allred[:, :], in_=cc_out[:, :])

            # final scalar math on [128, NT]
            sums_g = allred[:, 0:NT]
            texp_g = allred[:, NT:2 * NT]
            tln = big.tile([128, NT], f32, name="tln")
            nc.scalar.activation(out=tln[:, :], in_=texp_g, func=Act.Ln)
            tgt = big.tile([128, NT], f32, name="tgt")
            nc.vector.tensor_scalar_mul(tgt[:, :], tln[:, :], float(1.0 / S))
            t = big.tile([128, NT], f32, name="t")
            nc.vector.tensor_scalar(out=t[:, :], in0=tgt[:, :],
                                    scalar1=float(1.0 - EPS),
                                    scalar2=float(-1.0 + EPS),
                                    op0=Alu.min, op1=Alu.max)
            tsq = big.tile([128, NT], f32, name="tsq")
            nc.vector.tensor_mul(tsq[:, :], t[:, :], t[:, :])
            u = big.tile([128, NT], f32, name="u")
            nc.vector.tensor_scalar(out=u[:, :], in0=tsq[:, :],
                                    scalar1=-1.0, scalar2=1.0,
                                    op0=Alu.mult, op1=Alu.add)
            r = big.tile([128, NT], f32, name="r")
            nc.scalar.activation(out=r[:, :], in_=u[:, :], func=Act.Sqrt)
            rs = big.tile([128, NT], f32, name="rs")
            nc.vector.tensor_scalar_mul(rs[:, :], r[:, :],
                                        float(S * np.sin(MARGIN)))
            numer = big.tile([128, NT], f32, name="numer")
            nc.vector.scalar_tensor_tensor(
                out=numer[:, :], in0=t[:, :], scalar=float(S * np.cos(MARGIN)),
                in1=rs[:, :], op0=Alu.mult, op1=Alu.subtract)
            en = big.tile([128, NT], f32, name="en")
            nc.scalar.activation(out=en[:, :], in_=numer[:, :], func=Act.Exp)
            d1 = big.tile([128, NT], f32, name="d1")
            nc.vector.tensor_add(d1[:, :], en[:, :], sums_g)
            denom = big.tile([128, NT], f32, name="denom")
            nc.vector.tensor_sub(denom[:, :], d1[:, :], texp_g)
            ld = big.tile([128, NT], f32, name="ld")
            nc.scalar.activation(out=ld[:, :], in_=denom[:, :], func=Act.Ln)
            L = big.tile([128, NT], f32, name="L")
            nc.vector.tensor_sub(L[:, :], numer[:, :], ld[:, :])
            Lrow = big.tile([128, 1], f32, name="Lrow")
            nc.vector.tensor_reduce(out=Lrow[:, :], in_=L[:, :],
                                    axis=mybir.AxisListType.X, op=Alu.add)
            acc = psum.tile([1, 1], f32, tag="mm", name="acc")
            nc.tensor.matmul(acc[:, :], lhsT=Lrow[:, :], rhs=ones[:, :])
            fin = big.tile([1, 1], f32, name="fin")
            nc.scalar.activation(out=fin[:, :], in_=acc[:, :], func=Act.Copy,
                                 scale=float(-1.0 / N))
            nc.sync.dma_start(out=out_d[:, :], in_=fin[:, :])

    nc.compile()
    return nc


def _get_nc():
    if "nc" not in _COMPILED:
        _COMPILED["nc"] = _build()
    return _COMPILED["nc"]


def make_in_maps(x, labels, W):
    x = np.asarray(x, np.float32)
    labels = np.asarray(labels, np.int64)
    W = np.asarray(W, np.float32)

    # xt packed [128, 4N]: col d2*2N + dd*N + n <- x[n, (2*d2+dd)*128 + p]
    xtr = x.T.reshape(4, 128, N)                  # [drow, p, n]
    xt = np.ascontiguousarray(
        xtr.reshape(2, 2, 128, N).transpose(2, 0, 1, 3).reshape(128, 4 * N)
    ).astype(ml_dtypes.float8_e4m3)
    # xn packed [128, NT*D]: col i*D + d <- x[i*128 + p, d]
    xn = np.ascontiguousarray(
        x.reshape(NT, 128, D).transpose(1, 0, 2).reshape(128, NT * D)
    ).astype(ml_dtypes.bfloat16)

    in_maps = []
    for k in range(NCORES):
        lo = k * CLOC
        wtt = W[lo:lo + CLOC].T                   # [D, CLOC] = [drow*? , c]
        blocks = []
        for (c0, cw) in JC:
            blk = wtt[:, c0:c0 + cw].reshape(2, 2, 128, cw)  # [d2, dd, p, c]
            blocks.append(blk.transpose(0, 2, 1, 3).reshape(2, 128, 2 * cw))
        wt = np.ascontiguousarray(
            np.concatenate([b for blk2 in blocks for b in blk2], axis=1)
        ).astype(ml_dtypes.float8_e4m3)
        ll = labels - lo
        ll = np.where((ll >= 0) & (ll < CLOC), ll, -1).astype(np.float32)
        lab = np.ascontiguousarray(ll.reshape(NT, 128).T)  # [128, NT]
        in_maps.append({"xt": xt, "xn": xn, "wt": wt, "lab": lab})
    return in_maps


def kernel(x, labels, W, _trace=False, _trace_kwargs=None):
    from concourse.bass_utils import run_bass_kernel_spmd

    nc = _get_nc()
    in_maps = make_in_maps(x, labels, W)
    res = run_bass_kernel_spmd(nc, in_maps, core_ids=list(range(NCORES)),
                               trace=_trace, **(_trace_kwargs or {}))
    if _trace:
        _COMPILED["last_result"] = res
    out = np.asarray(res.results[0]["out"], np.float32).reshape(())
    return out
